# revision 11
# baseline (speedup 1.0000x reference)
"""Conditional VQ embedding forward on 8 trn2 NeuronCores — fp16-hi scheme.

Data-parallel over batch (4 per core). Score s[n,k] = z_n . e_k computed at a
common 2^20 product scale in ONE PSUM accumulation per n-tile:
  hi : fp16(z*2^10) . fp16(e*2^10)     2 fp16 matmuls (products exact in the
                                       PE's e10m23 datapath; 11 bits each side)
  c1 : e4m3(dz*2^12) . e4m3(e*2^8)     1 fp8 DoubleRow matmul (0.5 cyc/row)
  c2 : e4m3(z) . e4m3(de*2^20)         1 more;  dz = z - fp16(z), de likewise
Residuals are 2^-12-scale so single e4m3 pieces suffice: total score error
~1e-8, same as an exact 3-pass bf16 hi/lo kernel (1 flip in 131072).
v[n,k] = fp32(ps*2^-19 - A_n) on ACT: single fp32 rounding == the reference's
rounding grid, whose ties (first index wins) are load-bearing.
Pool (gpsimd) packs w = fp16((v + A_n)*2^15) — exact by cancellation — so the
DVE max8/max_index scans run at 2x. Device emits u32 indices only; the host
gathers fp32 codewords and replicates the straight-through arithmetic exactly.
"""

import numpy as np

B, D, HW, K = 32, 256, 4096, 512
NCORES, BPC = 8, 4
P = 128
NT = HW // P

GW = 2048
NG = HW // GW
TPG = GW // P

TRACE = False
LAST_RESULT = None
_NC_CACHE = {}


def _build():
    from contextlib import ExitStack

    import concourse.mybir as mybir
    from concourse import bacc
    from concourse.tile import TileContext

    f32 = mybir.dt.float32
    f16 = mybir.dt.float16
    f8 = mybir.dt.float8e4
    u32 = mybir.dt.uint32
    DR = mybir.MatmulPerfMode.DoubleRow
    Alu = mybir.AluOpType

    nc = bacc.Bacc("TRN2", target_bir_lowering=False, debug=False, num_devices=NCORES)
    zh16_in = nc.dram_tensor("zh16", [BPC, D, HW], f16, kind="ExternalInput")
    dz8_in = nc.dram_tensor("dz8", [BPC, D, HW], f8, kind="ExternalInput")
    z8_in = nc.dram_tensor("z8", [BPC, D, HW], f8, kind="ExternalInput")
    e16_in = nc.dram_tensor("e16", [BPC, D, K], f16, kind="ExternalInput")
    e8_in = nc.dram_tensor("e8", [BPC, D, 2 * K], f8, kind="ExternalInput")
    an_in = nc.dram_tensor("an", [BPC, HW], f32, kind="ExternalInput")
    idx_out = nc.dram_tensor("idx", [BPC, P, NT], u32, kind="ExternalOutput")

    with TileContext(nc) as tc, ExitStack() as ctx:
        cb_p = ctx.enter_context(tc.tile_pool(name="cbp", bufs=2))
        an_p = ctx.enter_context(tc.tile_pool(name="anp", bufs=2))
        z_p = ctx.enter_context(tc.tile_pool(name="zp", bufs=2))
        v_p = ctx.enter_context(tc.tile_pool(name="vp", bufs=4))
        w_p = ctx.enter_context(tc.tile_pool(name="wp", bufs=4))
        m_p = ctx.enter_context(tc.tile_pool(name="mp", bufs=8))
        i_p = ctx.enter_context(tc.tile_pool(name="ip", bufs=2))
        ps_p = ctx.enter_context(tc.tile_pool(name="psp", bufs=6, space="PSUM"))

        for b in range(BPC):
            e16_t = cb_p.tile([P, 2, K], f16, tag="e16")
            nc.sync.dma_start(e16_t[:], e16_in[b].rearrange("(c p) k -> p c k", p=P))
            e8_t = cb_p.tile([P, 2, 2 * K], f8, tag="e8")
            nc.sync.dma_start(e8_t[:], e8_in[b].rearrange("(c p) k -> p c k", p=P))
            an_all = an_p.tile([P, NT], f32, tag="an")
            nc.sync.dma_start(an_all[:], an_in[b, :].rearrange("(t p) -> p t", p=P))

            ig = i_p.tile([P, NT, 8], u32, tag="ig")
            for g in range(NG):
                gs = slice(g * GW, (g + 1) * GW)
                zh16_g = z_p.tile([P, 2, GW], f16, tag="zh16")
                nc.sync.dma_start(zh16_g[:], zh16_in[b, :, gs].rearrange("(c p) n -> p c n", p=P))
                dz8_g = z_p.tile([P, 2, GW], f8, tag="dz8")
                nc.sync.dma_start(dz8_g[:], dz8_in[b, :, gs].rearrange("(c p) n -> p c n", p=P))
                z8_g = z_p.tile([P, 2, GW], f8, tag="z8")
                nc.sync.dma_start(z8_g[:], z8_in[b, :, gs].rearrange("(c p) n -> p c n", p=P))
                for u in range(TPG):
                    t = g * TPG + u
                    us = slice(u * P, (u + 1) * P)

                    ps = ps_p.tile([P, K], f32, space="PSUM", tag="ps")
                    nc.tensor.matmul(ps[:], lhsT=zh16_g[:, 0, us], rhs=e16_t[:, 0, :], start=True, stop=False)
                    nc.tensor.matmul(ps[:], lhsT=zh16_g[:, 1, us], rhs=e16_t[:, 1, :], start=False, stop=False)
                    nc.tensor.matmul(ps[:], lhsT=dz8_g[:, :, us], rhs=e8_t[:, :, 0:K],
                                     perf_mode=DR, start=False, stop=False)
                    nc.tensor.matmul(ps[:], lhsT=z8_g[:, :, us], rhs=e8_t[:, :, K : 2 * K],
                                     perf_mode=DR, start=False, stop=True)

                    v = v_p.tile([P, K], f32, tag="v")
                    nc.scalar.activation(
                        out=v[:], in_=ps[:],
                        func=mybir.ActivationFunctionType.Identity,
                        bias=an_all[:, t : t + 1], scale=float(2.0 ** -19),
                    )
                    w = w_p.tile([P, K], f16, tag="w")
                    nc.gpsimd.tensor_scalar(
                        out=w[:], in0=v[:],
                        scalar1=an_all[:, t : t + 1], scalar2=32768.0,
                        op0=Alu.subtract, op1=Alu.mult,
                    )
                    m8 = m_p.tile([P, 8], f16, tag="m8")
                    nc.vector.max(out=m8[:], in_=w[:])
                    nc.vector.max_index(out=ig[:, t, :], in_max=m8[:], in_values=w[:])

            nc.sync.dma_start(out=idx_out[b, :, :], in_=ig[:, :, 0])

    nc.compile()
    return nc


def _get_nc():
    if "nc" not in _NC_CACHE:
        _NC_CACHE["nc"] = _build()
    return _NC_CACHE["nc"]


def kernel(z_e_x, C, weight):
    global LAST_RESULT
    import ml_dtypes
    from concourse.bass_utils import run_bass_kernel_spmd

    f8 = ml_dtypes.float8_e4m3

    z_e_x = np.asarray(z_e_x, dtype=np.float32)
    C = np.asarray(C).astype(np.int64)
    weight = np.asarray(weight, dtype=np.float32)

    import jax.numpy as jnp

    zj = jnp.asarray(z_e_x)
    zr_j = jnp.transpose(zj, (0, 2, 3, 1)).reshape(B, HW, D)
    A = jnp.sum(zr_j * zr_j, axis=-1, keepdims=True)
    an = -np.asarray(A)[..., 0]  # [B, HW] fp32

    zflat = z_e_x.reshape(B, D, HW)
    zh16 = (zflat * 2.0**10).astype(np.float16)
    dz = zflat - zh16.astype(np.float32) * 2.0**-10  # exact fp32 residual
    dz8 = (dz * 2.0**12).astype(f8)
    z8 = zflat.astype(f8)

    cb_all = weight[C]  # [B, K, D] fp32
    e16 = (cb_all * 2.0**10).astype(np.float16)
    de = cb_all - e16.astype(np.float32) * 2.0**-10
    e8b = (cb_all * 2.0**8).astype(f8)
    de8 = (de * 2.0**20).astype(f8)
    e16T = np.ascontiguousarray(np.swapaxes(e16, 1, 2))  # [B, D, K]
    e8T = np.ascontiguousarray(
        np.concatenate([np.swapaxes(e8b, 1, 2), np.swapaxes(de8, 1, 2)], axis=2)
    )  # [B, D, 2K]

    nc = _get_nc()
    in_maps = []
    for c in range(NCORES):
        bs = slice(c * BPC, (c + 1) * BPC)
        in_maps.append(
            dict(
                zh16=np.ascontiguousarray(zh16[bs]),
                dz8=np.ascontiguousarray(dz8[bs]),
                z8=np.ascontiguousarray(z8[bs]),
                e16=e16T[bs],
                e8=e8T[bs],
                an=np.ascontiguousarray(an[bs]).astype(np.float32),
            )
        )
    res = run_bass_kernel_spmd(nc, in_maps, core_ids=list(range(NCORES)), trace=TRACE)
    LAST_RESULT = res
    idx = np.concatenate(
        [np.asarray(r["idx"]).transpose(0, 2, 1).reshape(BPC, HW) for r in res.results], 0
    )  # [B, HW] u32

    zr = np.ascontiguousarray(z_e_x.transpose(0, 2, 3, 1)).reshape(B, HW, D)
    quant = np.take_along_axis(cb_all, idx.astype(np.int64)[:, :, None], axis=1)
    z_q = zr + (quant - zr)
    z_q_x = np.ascontiguousarray(z_q.reshape(B, 64, 64, D).transpose(0, 3, 1, 2))
    z_q_x_bar = np.ascontiguousarray(quant.reshape(B, 64, 64, D).transpose(0, 3, 1, 2))
    return z_q_x, z_q_x_bar


# revision 13
# speedup vs baseline: 7.3371x; 7.3371x over previous
"""Conditional VQ embedding forward on 8 trn2 NeuronCores.

Data-parallel over batch (4 per core). Score s[n,k] = z_n . e_k computed at a
common 2^20 product scale in ONE PSUM accumulation per 128-position n-tile:
  hi : fp16(z*2^10) . fp16(e*2^10)   2 fp16 matmuls (products exact in the
                                     PE's e10m23 datapath; 11 bits per side)
  c1 : e4m3(dz*2^12) . e4m3(e*2^8)   1 fp8 DoubleRow matmul (0.5 cyc/row)
  c2 : e4m3(z) . e4m3(de*2^20)       1 more;  dz = z - fp16(z), de likewise
Total score error ~5e-8 -> ~7 of 131072 argmin picks flip vs the fp32
reference (verified on HW; tolerance allows ~330 flips).

The argmin itself runs as ONE custom DVE instruction per tile, straight from
PSUM (custom-DVE Spec, registered at runtime):
  body  = ((Src0 - C0) + C0) - Idx * 2^-9,  C0 = A_n * 2^19 per partition
  accum = MAX
(Src0 - C0) is the fp32 rounding of 2^19*(2s - A_n) - the same relative
grid as the reference's fp32(dist), whose ties (first index wins) decide ~2%
of picks; + C0 cancels exactly (Sterbenz), leaving 2^19*(v+A): a multiple of
4 with magnitude <= ~8232, so the Idx*2^-9 tie-break rides in exact low bits
and MAX implements argmax-with-first-index-ties. The host decodes
k = -round(512*U) mod 2048 from the per-position fold value, gathers the
fp32 codewords, and replicates the straight-through arithmetic exactly.
"""

import numpy as np

B, D, HW, K = 32, 256, 4096, 512
NCORES, BPC = 8, 4
P = 128
NT = HW // P

GW = 2048
NG = HW // GW
TPG = GW // P

TRACE = False
LAST_RESULT = None
_NC_CACHE = {}


def _register_vq_op():
    from concourse import dve_ops
    from concourse.dve_spec import Spec, Src0, C0, C1, Idx, lower, AluOp
    from concourse.dve_uop import DveOpSpec
    from concourse.dve_ops import has_src1

    for o in dve_ops.OPS:
        if o.name == "VQ_ARGMAX_PACK":
            return o

    def _ref(in0, s0, s1):
        t1 = np.float32(np.float32(in0) - np.float32(s0))
        t2 = t1 + np.float32(s0)
        return t2 - np.arange(in0.shape[-1], dtype=np.float32) * np.float32(s1)

    body = ((Src0 - C0) + C0) - Idx * C1
    spec = Spec(body=body, accum=AluOp.MAX, reference=_ref)
    name = "VQ_ARGMAX_PACK"
    row = dve_ops._CUSTOM_DVE_ROW_BASE + len(dve_ops.OPS)
    shas = {}
    for ver in ("v3", "v4"):
        tmp = DveOpSpec(name=name, opcode=row, uops=lower(spec, ver=ver),
                        rd1_en=has_src1(spec))
        shas[ver] = tmp.sha(ver)
    op = dve_ops.DveOp(name, spec, subdim=False, uops_sha=shas)
    dve_ops.OPS.append(op)
    dve_ops._SUB_OPCODE_FOR_NAME[name] = row
    return op


def _build():
    from contextlib import ExitStack

    import concourse.mybir as mybir
    from concourse import bacc
    from concourse.tile import TileContext

    vq_op = _register_vq_op()

    f32 = mybir.dt.float32
    f16 = mybir.dt.float16
    f8 = mybir.dt.float8e4
    DR = mybir.MatmulPerfMode.DoubleRow

    nc = bacc.Bacc("TRN2", target_bir_lowering=False, debug=False, num_devices=NCORES)
    zh16_in = nc.dram_tensor("zh16", [BPC, D, HW], f16, kind="ExternalInput")
    dz8_in = nc.dram_tensor("dz8", [BPC, D, HW], f8, kind="ExternalInput")
    z8_in = nc.dram_tensor("z8", [BPC, D, HW], f8, kind="ExternalInput")
    e16_in = nc.dram_tensor("e16", [BPC, D, K], f16, kind="ExternalInput")
    e8_in = nc.dram_tensor("e8", [BPC, D, 2 * K], f8, kind="ExternalInput")
    a19_in = nc.dram_tensor("a19", [BPC, HW], f32, kind="ExternalInput")
    u_out = nc.dram_tensor("u", [BPC, P, NT], f32, kind="ExternalOutput")

    with TileContext(nc) as tc, ExitStack() as ctx:
        cb_p = ctx.enter_context(tc.tile_pool(name="cbp", bufs=2))
        an_p = ctx.enter_context(tc.tile_pool(name="anp", bufs=2))
        z_p = ctx.enter_context(tc.tile_pool(name="zp", bufs=2))
        w_p = ctx.enter_context(tc.tile_pool(name="wp", bufs=3))
        o_p = ctx.enter_context(tc.tile_pool(name="op", bufs=2))
        ps_p = ctx.enter_context(tc.tile_pool(name="psp", bufs=6, space="PSUM"))

        for b in range(BPC):
            e16_t = cb_p.tile([P, 2, K], f16, tag="e16")
            nc.sync.dma_start(e16_t[:], e16_in[b].rearrange("(c p) k -> p c k", p=P))
            e8_t = cb_p.tile([P, 2, 2 * K], f8, tag="e8")
            nc.sync.dma_start(e8_t[:], e8_in[b].rearrange("(c p) k -> p c k", p=P))
            a19_all = an_p.tile([P, NT], f32, tag="a19")
            nc.sync.dma_start(a19_all[:], a19_in[b, :].rearrange("(t p) -> p t", p=P))

            og = o_p.tile([P, NT], f32, tag="og")
            for g in range(NG):
                gs = slice(g * GW, (g + 1) * GW)
                zh16_g = z_p.tile([P, 2, GW], f16, tag="zh16")
                nc.sync.dma_start(zh16_g[:], zh16_in[b, :, gs].rearrange("(c p) n -> p c n", p=P))
                dz8_g = z_p.tile([P, 2, GW], f8, tag="dz8")
                nc.sync.dma_start(dz8_g[:], dz8_in[b, :, gs].rearrange("(c p) n -> p c n", p=P))
                z8_g = z_p.tile([P, 2, GW], f8, tag="z8")
                nc.sync.dma_start(z8_g[:], z8_in[b, :, gs].rearrange("(c p) n -> p c n", p=P))
                for u in range(TPG):
                    t = g * TPG + u
                    us = slice(u * P, (u + 1) * P)

                    ps = ps_p.tile([P, K], f32, space="PSUM", tag="ps")
                    nc.tensor.matmul(ps[:], lhsT=zh16_g[:, 0, us], rhs=e16_t[:, 0, :], start=True, stop=False)
                    nc.tensor.matmul(ps[:], lhsT=zh16_g[:, 1, us], rhs=e16_t[:, 1, :], start=False, stop=False)
                    nc.tensor.matmul(ps[:], lhsT=dz8_g[:, :, us], rhs=e8_t[:, :, 0:K],
                                     perf_mode=DR, start=False, stop=False)
                    nc.tensor.matmul(ps[:], lhsT=z8_g[:, :, us], rhs=e8_t[:, :, K : 2 * K],
                                     perf_mode=DR, start=False, stop=True)

                    scr = w_p.tile([P, K], f16, tag="scr")
                    nc.vector._custom_dve(
                        vq_op,
                        out=scr[:],
                        accum_out=og[:, t : t + 1],
                        in0=ps[:],
                        s0=a19_all[:, t : t + 1],
                        s1=float(2.0 ** -9),
                    )

            nc.sync.dma_start(out=u_out[b, :, :], in_=og[:])

    nc.compile()
    return nc


def _get_nc():
    if "nc" not in _NC_CACHE:
        _NC_CACHE["nc"] = _build()
    return _NC_CACHE["nc"]


def kernel(z_e_x, C, weight):
    global LAST_RESULT
    import ml_dtypes
    from concourse.bass_utils import run_bass_kernel_spmd

    f8 = ml_dtypes.float8_e4m3

    z_e_x = np.asarray(z_e_x, dtype=np.float32)
    C = np.asarray(C).astype(np.int64)
    weight = np.asarray(weight, dtype=np.float32)

    import jax.numpy as jnp

    zj = jnp.asarray(z_e_x)
    zr_j = jnp.transpose(zj, (0, 2, 3, 1)).reshape(B, HW, D)
    A = jnp.sum(zr_j * zr_j, axis=-1, keepdims=True)
    a19 = np.asarray(A)[..., 0] * np.float32(2.0 ** 19)  # [B, HW] fp32, exact scale

    zflat = z_e_x.reshape(B, D, HW)
    zh16 = (zflat * 2.0**10).astype(np.float16)
    dz = zflat - zh16.astype(np.float32) * 2.0**-10  # exact fp32 residual
    dz8 = (dz * 2.0**12).astype(f8)
    z8 = zflat.astype(f8)

    cb_all = weight[C]  # [B, K, D] fp32
    e16 = (cb_all * 2.0**10).astype(np.float16)
    de = cb_all - e16.astype(np.float32) * 2.0**-10
    e8b = (cb_all * 2.0**8).astype(f8)
    de8 = (de * 2.0**20).astype(f8)
    e16T = np.ascontiguousarray(np.swapaxes(e16, 1, 2))  # [B, D, K]
    e8T = np.ascontiguousarray(
        np.concatenate([np.swapaxes(e8b, 1, 2), np.swapaxes(de8, 1, 2)], axis=2)
    )  # [B, D, 2K]

    nc = _get_nc()
    in_maps = []
    for c in range(NCORES):
        bs = slice(c * BPC, (c + 1) * BPC)
        in_maps.append(
            dict(
                zh16=np.ascontiguousarray(zh16[bs]),
                dz8=np.ascontiguousarray(dz8[bs]),
                z8=np.ascontiguousarray(z8[bs]),
                e16=e16T[bs],
                e8=e8T[bs],
                a19=np.ascontiguousarray(a19[bs]).astype(np.float32),
            )
        )
    res = run_bass_kernel_spmd(nc, in_maps, core_ids=list(range(NCORES)), trace=TRACE)
    LAST_RESULT = res
    # device emits max_k of 2^19*(v+A) - k*2^-9 per position ([BPC, P, NT],
    # n = t*P + p); the winning k is recovered from the exact low bits
    uf = np.concatenate(
        [np.asarray(r["u"]).transpose(0, 2, 1).reshape(BPC, HW) for r in res.results], 0
    ).astype(np.float64)  # [B, HW]
    idx = np.minimum(np.mod(-np.rint(uf * 512.0).astype(np.int64), 2048), K - 1)

    zr = np.ascontiguousarray(z_e_x.transpose(0, 2, 3, 1)).reshape(B, HW, D)
    quant = np.take_along_axis(cb_all, idx[:, :, None], axis=1)
    z_q = zr + (quant - zr)
    z_q_x = np.ascontiguousarray(z_q.reshape(B, 64, 64, D).transpose(0, 3, 1, 2))
    z_q_x_bar = np.ascontiguousarray(quant.reshape(B, 64, 64, D).transpose(0, 3, 1, 2))
    return z_q_x, z_q_x_bar


# revision 15
# speedup vs baseline: 7.5064x; 1.0231x over previous
"""Conditional VQ embedding forward on 8 trn2 NeuronCores.

Data-parallel over batch (4 per core). Score s[n,k] = z_n . e_k computed at a
common 2^20 product scale in ONE PSUM accumulation per 128-position n-tile:
  hi : fp16(z*2^10) . fp16(e*2^10)   2 fp16 matmuls (products exact in the
                                     PE's e10m23 datapath; 11 bits per side)
  c1 : e4m3(dz*2^12) . e4m3(e*2^8)   1 fp8 DoubleRow matmul (0.5 cyc/row)
  c2 : e4m3(z) . e4m3(de*2^20)       1 more;  dz = z - fp16(z), de likewise
Total score error ~5e-8 -> ~7 of 131072 argmin picks flip vs the fp32
reference (verified on HW; tolerance allows ~330 flips).

The argmin itself runs as ONE custom DVE instruction per tile, straight from
PSUM (custom-DVE Spec, registered at runtime):
  body  = ((Src0 - C0) + C0) - Idx * 2^-9,  C0 = A_n * 2^19 per partition
  accum = MAX
(Src0 - C0) is the fp32 rounding of 2^19*(2s - A_n) - the same relative
grid as the reference's fp32(dist), whose ties (first index wins) decide ~2%
of picks; + C0 cancels exactly (Sterbenz), leaving 2^19*(v+A): a multiple of
4 with magnitude <= ~8232, so the Idx*2^-9 tie-break rides in exact low bits
and MAX implements argmax-with-first-index-ties. The host decodes
k = -round(512*U) mod 2048 from the per-position fold value, gathers the
fp32 codewords, and replicates the straight-through arithmetic exactly.
"""

import numpy as np

B, D, HW, K = 32, 256, 4096, 512
NCORES, BPC = 8, 4
P = 128
NT = HW // P

GW = 2048
NG = HW // GW
TPG = GW // P

TRACE = False
LAST_RESULT = None
_NC_CACHE = {}


def _register_vq_op():
    from concourse import dve_ops
    from concourse.dve_spec import Spec, Src0, C0, C1, Idx, lower, AluOp
    from concourse.dve_uop import DveOpSpec
    from concourse.dve_ops import has_src1

    for o in dve_ops.OPS:
        if o.name == "VQ_ARGMAX_PACK":
            return o

    def _ref(in0, s0, s1):
        t1 = np.float32(np.float32(in0) - np.float32(s0))
        t2 = t1 + np.float32(s0)
        return t2 - np.arange(in0.shape[-1], dtype=np.float32) * np.float32(s1)

    body = ((Src0 - C0) + C0) - Idx * C1
    spec = Spec(body=body, accum=AluOp.MAX, reference=_ref)
    name = "VQ_ARGMAX_PACK"
    row = dve_ops._CUSTOM_DVE_ROW_BASE + len(dve_ops.OPS)
    shas = {}
    for ver in ("v3", "v4"):
        tmp = DveOpSpec(name=name, opcode=row, uops=lower(spec, ver=ver),
                        rd1_en=has_src1(spec))
        shas[ver] = tmp.sha(ver)
    op = dve_ops.DveOp(name, spec, subdim=False, uops_sha=shas)
    dve_ops.OPS.append(op)
    dve_ops._SUB_OPCODE_FOR_NAME[name] = row
    return op


def _build():
    from contextlib import ExitStack

    import concourse.mybir as mybir
    from concourse import bacc
    from concourse.tile import TileContext

    vq_op = _register_vq_op()

    f32 = mybir.dt.float32
    f16 = mybir.dt.float16
    f8 = mybir.dt.float8e4
    DR = mybir.MatmulPerfMode.DoubleRow

    nc = bacc.Bacc("TRN2", target_bir_lowering=False, debug=False, num_devices=NCORES)
    zh16_in = nc.dram_tensor("zh16", [BPC, D, HW], f16, kind="ExternalInput")
    dz8_in = nc.dram_tensor("dz8", [BPC, D, HW], f8, kind="ExternalInput")
    z8_in = nc.dram_tensor("z8", [BPC, D, HW], f8, kind="ExternalInput")
    e16_in = nc.dram_tensor("e16", [BPC, D, K], f16, kind="ExternalInput")
    e8_in = nc.dram_tensor("e8", [BPC, D, 2 * K], f8, kind="ExternalInput")
    a19_in = nc.dram_tensor("a19", [BPC, HW], f32, kind="ExternalInput")
    u_out = nc.dram_tensor("u", [BPC, P, NT], f32, kind="ExternalOutput")

    with TileContext(nc) as tc, ExitStack() as ctx:
        cb_p = ctx.enter_context(tc.tile_pool(name="cbp", bufs=2))
        an_p = ctx.enter_context(tc.tile_pool(name="anp", bufs=2))
        z_p = ctx.enter_context(tc.tile_pool(name="zp", bufs=2))
        w_p = ctx.enter_context(tc.tile_pool(name="wp", bufs=3))
        o_p = ctx.enter_context(tc.tile_pool(name="op", bufs=2))
        ps_p = ctx.enter_context(tc.tile_pool(name="psp", bufs=6, space="PSUM"))

        for b in range(BPC):
            # issue order puts the first matmul's deps (e16 + zh16 chunk 0)
            # at the front of the DMA queue: ~770KB instead of ~2.8MB
            e16_t = cb_p.tile([P, 2, K], f16, tag="e16")
            nc.sync.dma_start(e16_t[:], e16_in[b].rearrange("(c p) k -> p c k", p=P))
            og = o_p.tile([P, NT], f32, tag="og")
            e8_t = None
            a19_all = None
            for g in range(NG):
                gs = slice(g * GW, (g + 1) * GW)
                zh16_g = z_p.tile([P, 2, GW], f16, tag="zh16")
                if g == 0:
                    nc.sync.dma_start(zh16_g[:, 0:1, :], zh16_in[b, 0:P, gs].rearrange("(c p) n -> p c n", p=P))
                    nc.sync.dma_start(zh16_g[:, 1:2, :], zh16_in[b, P : 2 * P, gs].rearrange("(c p) n -> p c n", p=P))
                else:
                    nc.sync.dma_start(zh16_g[:], zh16_in[b, :, gs].rearrange("(c p) n -> p c n", p=P))
                if e8_t is None:
                    e8_t = cb_p.tile([P, 2, 2 * K], f8, tag="e8")
                    nc.sync.dma_start(e8_t[:], e8_in[b].rearrange("(c p) k -> p c k", p=P))
                dz8_g = z_p.tile([P, 2, GW], f8, tag="dz8")
                nc.sync.dma_start(dz8_g[:], dz8_in[b, :, gs].rearrange("(c p) n -> p c n", p=P))
                z8_g = z_p.tile([P, 2, GW], f8, tag="z8")
                nc.sync.dma_start(z8_g[:], z8_in[b, :, gs].rearrange("(c p) n -> p c n", p=P))
                if a19_all is None:
                    a19_all = an_p.tile([P, NT], f32, tag="a19")
                    nc.sync.dma_start(a19_all[:], a19_in[b, :].rearrange("(t p) -> p t", p=P))
                for u in range(TPG):
                    t = g * TPG + u
                    us = slice(u * P, (u + 1) * P)

                    ps = ps_p.tile([P, K], f32, space="PSUM", tag="ps")
                    nc.tensor.matmul(ps[:], lhsT=zh16_g[:, 0, us], rhs=e16_t[:, 0, :], start=True, stop=False)
                    nc.tensor.matmul(ps[:], lhsT=zh16_g[:, 1, us], rhs=e16_t[:, 1, :], start=False, stop=False)
                    nc.tensor.matmul(ps[:], lhsT=dz8_g[:, :, us], rhs=e8_t[:, :, 0:K],
                                     perf_mode=DR, start=False, stop=False)
                    nc.tensor.matmul(ps[:], lhsT=z8_g[:, :, us], rhs=e8_t[:, :, K : 2 * K],
                                     perf_mode=DR, start=False, stop=True)

                    scr = w_p.tile([P, K], f16, tag="scr")
                    nc.vector._custom_dve(
                        vq_op,
                        out=scr[:],
                        accum_out=og[:, t : t + 1],
                        in0=ps[:],
                        s0=a19_all[:, t : t + 1],
                        s1=float(2.0 ** -9),
                    )

            nc.sync.dma_start(out=u_out[b, :, :], in_=og[:])

    nc.compile()
    return nc


def _get_nc():
    if "nc" not in _NC_CACHE:
        _NC_CACHE["nc"] = _build()
    return _NC_CACHE["nc"]


def kernel(z_e_x, C, weight):
    global LAST_RESULT
    import ml_dtypes
    from concourse.bass_utils import run_bass_kernel_spmd

    f8 = ml_dtypes.float8_e4m3

    z_e_x = np.asarray(z_e_x, dtype=np.float32)
    C = np.asarray(C).astype(np.int64)
    weight = np.asarray(weight, dtype=np.float32)

    import jax.numpy as jnp

    zj = jnp.asarray(z_e_x)
    zr_j = jnp.transpose(zj, (0, 2, 3, 1)).reshape(B, HW, D)
    A = jnp.sum(zr_j * zr_j, axis=-1, keepdims=True)
    a19 = np.asarray(A)[..., 0] * np.float32(2.0 ** 19)  # [B, HW] fp32, exact scale

    zflat = z_e_x.reshape(B, D, HW)
    zh16 = (zflat * 2.0**10).astype(np.float16)
    dz = zflat - zh16.astype(np.float32) * 2.0**-10  # exact fp32 residual
    dz8 = (dz * 2.0**12).astype(f8)
    z8 = zflat.astype(f8)

    cb_all = weight[C]  # [B, K, D] fp32
    e16 = (cb_all * 2.0**10).astype(np.float16)
    de = cb_all - e16.astype(np.float32) * 2.0**-10
    e8b = (cb_all * 2.0**8).astype(f8)
    de8 = (de * 2.0**20).astype(f8)
    e16T = np.ascontiguousarray(np.swapaxes(e16, 1, 2))  # [B, D, K]
    e8T = np.ascontiguousarray(
        np.concatenate([np.swapaxes(e8b, 1, 2), np.swapaxes(de8, 1, 2)], axis=2)
    )  # [B, D, 2K]

    nc = _get_nc()
    in_maps = []
    for c in range(NCORES):
        bs = slice(c * BPC, (c + 1) * BPC)
        in_maps.append(
            dict(
                zh16=np.ascontiguousarray(zh16[bs]),
                dz8=np.ascontiguousarray(dz8[bs]),
                z8=np.ascontiguousarray(z8[bs]),
                e16=e16T[bs],
                e8=e8T[bs],
                a19=np.ascontiguousarray(a19[bs]).astype(np.float32),
            )
        )
    res = run_bass_kernel_spmd(nc, in_maps, core_ids=list(range(NCORES)), trace=TRACE)
    LAST_RESULT = res
    # device emits max_k of 2^19*(v+A) - k*2^-9 per position ([BPC, P, NT],
    # n = t*P + p); the winning k is recovered from the exact low bits
    uf = np.concatenate(
        [np.asarray(r["u"]).transpose(0, 2, 1).reshape(BPC, HW) for r in res.results], 0
    ).astype(np.float64)  # [B, HW]
    idx = np.minimum(np.mod(-np.rint(uf * 512.0).astype(np.int64), 2048), K - 1)

    zr = np.ascontiguousarray(z_e_x.transpose(0, 2, 3, 1)).reshape(B, HW, D)
    quant = np.take_along_axis(cb_all, idx[:, :, None], axis=1)
    z_q = zr + (quant - zr)
    z_q_x = np.ascontiguousarray(z_q.reshape(B, 64, 64, D).transpose(0, 3, 1, 2))
    z_q_x_bar = np.ascontiguousarray(quant.reshape(B, 64, 64, D).transpose(0, 3, 1, 2))
    return z_q_x, z_q_x_bar
